# revision 19
# baseline (speedup 1.0000x reference)
"""Trainium2 Bass kernel for nn_ChannelAttentionModule (cyclic window mean +
channel attention). Self-contained: accepts FULL inputs, shards spatial dim
across 8 NeuronCores, returns FULL [64, 256] output.

Math: cyclic_window_mean over the batch axis is a matmul with a fixed [64,64]
window matrix M.  Per core (spatial shard of 512 of the 4096 positions):
  - stream x as [128, 2048] tiles (partitions = 2 spatial half-groups x 64 batch)
  - PE: y = blockdiag(M^T, M^T)^T @ x_tile  -> windowed means per position
  - DVE: running elementwise max over tiles    (-> spatial max of y)
  - PE: accumulate vstack(M^T, M^T)^T @ x_tile into one PSUM bank
        (-> spatial sum of y, partition halves pre-folded)
  - AllGather the packed [64, 512] (max | sum) partials across 8 cores
  - every core folds + computes the tiny MLP / softmax / final window matmul
"""

import os
import sys

import numpy as np

for _p in ("/opt/trn_rl_repo", "/root/.axon_site/_ro/trn_rl_repo"):
    if os.path.isdir(_p) and _p not in sys.path:
        sys.path.insert(0, _p)

import concourse.bass as bass
import concourse.mybir as mybir
import concourse.tile as tile
from concourse import bacc
from concourse import bass_utils as _bu
from concourse.bass_utils import run_bass_kernel_spmd

# Redundant-LDWEIGHTS elision: every streaming matmul reuses the same
# stationary weights, so let walrus's ldw-opt pass drop the reloads.
_orig_run_command = _bu.run_command

def _run_command_ldwopt(argv, **kwargs):
    argv = [a.replace("--enable-ldw-opt=false", "--enable-ldw-opt=true")
            if isinstance(a, str) else a for a in argv]
    return _orig_run_command(argv, **kwargs)

_bu.run_command = _run_command_ldwopt

B = 64          # batch
S = 64 * 64     # flattened spatial
C = 256         # channels
CE = 768        # hidden (C * 3)
NCORES = 8
S_CORE = S // NCORES   # 512 spatial positions per core
G = 32                 # positions per partition half-group per tile
TP = 2 * G             # 16 positions per tile
NT = S_CORE // TP      # 32 tiles per core
F = G * C              # 2048 free elements per tile
FQ = 512               # matmul moving free dim (one PSUM bank, fp32)
NQ = F // FQ           # 4 matmul chunks per tile
DT = mybir.dt.float32
AF = mybir.ActivationFunctionType


def _win_matrix(w: int) -> np.ndarray:
    """M such that cyclic_window_mean(x, w) == M @ x (along axis 0)."""
    m = np.zeros((B, B), np.float64)
    for i in range(B):
        if i >= w:
            m[i, i - w:i] = 1.0 / w
        else:
            m[i, : i + 1] = 1.0 / (w + 1)
            m[i, B - (w - i):] = 1.0 / (w + 1)
    return m.astype(np.float32)


def _build_program(wn: int) -> bass.Bass:
    nc = bacc.Bacc(
        "TRN2", target_bir_lowering=False, debug=False, num_devices=NCORES
    )

    DTR = mybir.dt.float32r  # fp32 bits, single-pass PE mode (1 cyc/row)
    xs = nc.declare_dram_parameter("xs", [B, S_CORE, C], DTR, isOutput=False)
    wblk_d = nc.declare_dram_parameter("wblk", [128, 128], DTR, isOutput=False)
    wstk_d = nc.declare_dram_parameter("wstk", [128, 64], DTR, isOutput=False)
    wfin_d = nc.declare_dram_parameter("wfin", [64, 64], DTR, isOutput=False)
    eye_d = nc.declare_dram_parameter("eye64", [64, 64], DT, isOutput=False)
    w1m_d = nc.declare_dram_parameter("w1m", [C, CE], DTR, isOutput=False)
    b1m_d = nc.declare_dram_parameter("b1m", [CE], DT, isOutput=False)
    w2m_d = nc.declare_dram_parameter("w2m", [CE, C], DTR, isOutput=False)
    b2m_d = nc.declare_dram_parameter("b2m", [1, C], DT, isOutput=False)
    w1a_d = nc.declare_dram_parameter("w1a", [C, CE], DTR, isOutput=False)
    b1a_d = nc.declare_dram_parameter("b1a", [CE], DT, isOutput=False)
    w2a_d = nc.declare_dram_parameter("w2a", [CE, C], DTR, isOutput=False)
    b2a_d = nc.declare_dram_parameter("b2a", [1, C], DT, isOutput=False)
    out_d = nc.declare_dram_parameter("out", [B, C], DT, isOutput=True)

    with tile.TileContext(nc) as tc:
        with (
            tc.tile_pool(name="const", bufs=1) as cpool,
            tc.tile_pool(name="x", bufs=3) as xpool,
            tc.tile_pool(name="pmax", bufs=2, space="PSUM") as pmax,
            tc.tile_pool(name="psum_sum", bufs=1, space="PSUM") as psump,
            tc.tile_pool(name="ptail", bufs=2, space="PSUM") as ptail,
            tc.tile_pool(name="pa_pool", bufs=1, space="PSUM") as papool,
            tc.tile_pool(name="sb", bufs=1) as spool,
            tc.tile_pool(name="dram", bufs=1, space="DRAM") as dpool,
        ):
            # ---- constants into SBUF (all 128-partition tiles: matmul
            #      operands must sit at base partition 0) ----
            wblk_sb = cpool.tile([128, 128], DTR, tag="wblk")
            nc.sync.dma_start(wblk_sb[:], wblk_d[:])
            wstk_sb = cpool.tile([128, 64], DTR, tag="wstk")
            nc.sync.dma_start(wstk_sb[:], wstk_d[:])
            wfin_sb = cpool.tile([128, 64], DTR, tag="wfin")
            nc.gpsimd.dma_start(wfin_sb[0:64, :], wfin_d[:])
            eye_sb = cpool.tile([128, 64], DT, tag="eye")
            nc.gpsimd.dma_start(eye_sb[0:64, :], eye_d[:])
            # W1 as [128, 2, CE]: partition = k-within-half, free = (half, n)
            w1m_sb = cpool.tile([128, 2, CE], DTR, tag="w1m")
            nc.gpsimd.dma_start(w1m_sb[:], w1m_d[:].rearrange("(h k) n -> k h n", h=2))
            w1a_sb = cpool.tile([128, 2, CE], DTR, tag="w1a")
            nc.gpsimd.dma_start(w1a_sb[:], w1a_d[:].rearrange("(h k) n -> k h n", h=2))
            # W2 as [128, 6, C]: partition = k-within-chunk, free = (chunk, n)
            w2m_sb = cpool.tile([128, 6, C], DTR, tag="w2m")
            nc.gpsimd.dma_start(w2m_sb[:], w2m_d[:].rearrange("(m k) n -> k m n", m=6))
            w2a_sb = cpool.tile([128, 6, C], DTR, tag="w2a")
            nc.gpsimd.dma_start(w2a_sb[:], w2a_d[:].rearrange("(m k) n -> k m n", m=6))
            # b1 transposed to [128, 6] -> per-partition bias for the ACT relu
            b1m_sb = cpool.tile([128, 6], DT, tag="b1m")
            nc.gpsimd.dma_start(b1m_sb[:], b1m_d[:].rearrange("(m k) -> k m", m=6))
            b1a_sb = cpool.tile([128, 6], DT, tag="b1a")
            nc.gpsimd.dma_start(b1a_sb[:], b1a_d[:].rearrange("(m k) -> k m", m=6))
            # row-0 smalls: ones[0:64] | b2m[64:320] | b2a[320:576]
            smalls = cpool.tile([128, 576], DT, tag="smalls")
            nc.vector.memset(smalls[0:1, 0:64], 1.0)
            nc.gpsimd.dma_start(smalls[0:1, 64:320], b2m_d[:])
            nc.gpsimd.dma_start(smalls[0:1, 320:576], b2a_d[:])
            ones_sb = smalls[0:1, 0:64]
            b2m_sb = smalls[0:1, 64:320]
            b2a_sb = smalls[0:1, 320:576]

            # ---- main streaming phase ----
            # Partition packing p = 2*b + h (h = spatial half-group INNER):
            # one DMA per tile covers all 128 partitions with a flat outer
            # batch dim, so descriptors spread over all 16 SDMA engines
            # (engine = partition/8; a 64-partition half engages only 8).
            max_acc = spool.tile([128, 2 * FQ], DT, tag="max_acc")
            nc.vector.memset(max_acc[:], -1e30)
            sum_ps = psump.tile([128, FQ], DT, tag="sum_ps")
            wblk_r = wblk_sb[:]
            wstk_r = wstk_sb[:]

            # s_local = t*TP + h*G + g ; partition = (b, h) ; free = (g, c)
            xs_r = xs[:].rearrange("b (t h g) c -> t b h (g c)", h=2, g=G)
            for t in range(NT):
                xt = xpool.tile([128, F], DTR, tag="xt")
                src_t = xs_r[t]
                nc.sync.dma_start(xt[:, 0:F // 2], src_t[:, :, 0:F // 2])
                nc.scalar.dma_start(xt[:, F // 2:F], src_t[:, :, F // 2:F])
                xt_r = xt[:]
                for half in range(F // (2 * FQ)):
                    ym = pmax.tile([128, 2 * FQ], DT, tag="ym")
                    for q in range(2):
                        col = (2 * half + q) * FQ
                        nc.tensor.matmul(
                            ym[:, q * FQ:(q + 1) * FQ], wblk_r,
                            xt_r[:, col:col + FQ], start=True, stop=True,
                        )
                    nc.vector.tensor_max(max_acc[:], max_acc[:], ym[:])
                    for q in range(2):
                        col = (2 * half + q) * FQ
                        first = (t == 0 and half == 0 and q == 0)
                        last = (t == NT - 1
                                and half == F // (2 * FQ) - 1 and q == 1)
                        nc.tensor.matmul(
                            sum_ps[0:64, :], wstk_r, xt_r[:, col:col + FQ],
                            start=first, stop=last, skip_group_check=True,
                        )

            # ---- fold local partials to [64, C] each, pack as [64, 2C] ----
            # partition fold pairs (2i, 2i+1): extract strided halves via DMA
            ev64 = spool.tile([128, 2 * FQ], DT, tag="ev64")
            od64 = spool.tile([128, 2 * FQ], DT, tag="od64")
            nc.sync.dma_start(ev64[0:64, :], max_acc[0:128:2, :])
            nc.scalar.dma_start(od64[0:64, :], max_acc[1:128:2, :])
            u = spool.tile([128, 2 * FQ], DT, tag="u")
            nc.vector.tensor_max(u[0:64, :], ev64[0:64, :], od64[0:64, :])
            nc.vector.tensor_max(u[0:64, 0:FQ], u[0:64, 0:FQ], u[0:64, FQ:2 * FQ])
            pk = spool.tile([128, 2 * C], DT, tag="pk")
            nc.vector.tensor_max(pk[0:64, 0:C], u[0:64, 0:C], u[0:64, C:2 * C])
            su = spool.tile([128, FQ], DT, tag="su")
            nc.scalar.copy(su[0:64, :], sum_ps[0:64, :])
            nc.vector.tensor_add(
                pk[0:64, C:2 * C], su[0:64, 0:C], su[0:64, C:2 * C]
            )

            # ---- cross-core combine: one AllGather + local fold ----
            rg = [list(range(NCORES))]
            gin = dpool.tile([64, 2 * C], DT, tag="gin")
            gout = dpool.tile([NCORES * 64, 2 * C], DT, tag="gout")
            nc.gpsimd.dma_start(gin[:], pk[0:64, :])
            nc.gpsimd.collective_compute(
                "AllGather", mybir.AluOpType.bypass, replica_groups=rg,
                ins=[gin.opt()], outs=[gout.opt()],
            )
            g_sb = spool.tile([128, NCORES, 2 * C], DT, tag="g_sb")
            nc.gpsimd.dma_start(
                g_sb[0:64, :, :], gout[:].rearrange("(r b) n -> b r n", r=NCORES)
            )
            mxf = spool.tile([128, C], DT, tag="mxf")
            svf = spool.tile([128, C], DT, tag="svf")
            nc.vector.tensor_copy(mxf[0:64, :], g_sb[0:64, 0, 0:C])
            nc.vector.tensor_copy(svf[0:64, :], g_sb[0:64, 0, C:2 * C])
            for r in range(1, NCORES):
                nc.vector.tensor_max(mxf[0:64, :], mxf[0:64, :], g_sb[0:64, r, 0:C])
                nc.vector.tensor_add(
                    svf[0:64, :], svf[0:64, :], g_sb[0:64, r, C:2 * C]
                )

            # ---- transpose [64, 256] -> [128, 2, 64] (chunked over C) ----
            def transpose_bc(src, tag):
                dst = spool.tile([128, 2, 64], DTR, tag=tag)
                for ch in range(2):
                    pt = ptail.tile([128, 64], DT, tag="ph")
                    nc.tensor.transpose(
                        pt[:], src[0:64, ch * 128:(ch + 1) * 128], eye_sb[0:64, :]
                    )
                    nc.scalar.copy(dst[:, ch, :], pt[:])
                return dst

            mxT = transpose_bc(mxf, "mxT")
            svT = transpose_bc(svf, "svT")

            # ---- the two tiny MLPs (relu(relu(v @ W1 + b1) @ W2 + b2)) ----
            def mlp(vT, w1_sb, b1_sb, w2_sb, b2_sb, tag):
                h1 = spool.tile([128, 6, 64], DTR, tag=f"h1_{tag}")
                for m in range(6):
                    ph = ptail.tile([128, 64], DT, tag="ph")
                    nc.tensor.matmul(
                        ph[:], w1_sb[:, 0, m * 128:(m + 1) * 128], vT[:, 0, :],
                        start=True, stop=False,
                    )
                    nc.tensor.matmul(
                        ph[:], w1_sb[:, 1, m * 128:(m + 1) * 128], vT[:, 1, :],
                        start=False, stop=True,
                    )
                    nc.scalar.activation(
                        h1[:, m, :], ph[:], AF.Relu, bias=b1_sb[:, m:m + 1]
                    )
                pa = papool.tile([128, C], DT, tag="pa")
                for m in range(6):
                    nc.tensor.matmul(
                        pa[0:64, :], h1[:, m, :], w2_sb[:, m, :],
                        start=(m == 0), stop=False,
                    )
                nc.tensor.matmul(
                    pa[0:64, :], ones_sb, b2_sb, start=False, stop=True
                )
                dst = spool.tile([128, C], DT, tag=f"mlp_{tag}")
                nc.scalar.activation(dst[0:64, :], pa[0:64, :], AF.Relu)
                return dst

            m_sb = mlp(mxT, w1m_sb, b1m_sb, w2m_sb, b2m_sb, "m")
            a_sb = mlp(svT, w1a_sb, b1a_sb, w2a_sb, b2a_sb, "a")

            # ---- sigmoid(m + a), softmax over channels ----
            t_sb = spool.tile([128, C], DT, tag="t_sb")
            nc.vector.tensor_add(t_sb[0:64, :], m_sb[0:64, :], a_sb[0:64, :])
            s_sb = spool.tile([128, C], DT, tag="s_sb")
            nc.scalar.activation(s_sb[0:64, :], t_sb[0:64, :], AF.Sigmoid)
            red = spool.tile([128, 4], DT, tag="red")  # rsum | rinv
            e_sb = spool.tile([128, C], DT, tag="e_sb")
            nc.scalar.activation(e_sb[0:64, :], s_sb[0:64, :], AF.Exp)
            nc.vector.tensor_reduce(
                red[0:64, 1:2], e_sb[0:64, :], axis=mybir.AxisListType.X,
                op=mybir.AluOpType.add,
            )
            nc.vector.reciprocal(red[0:64, 2:3], red[0:64, 1:2])
            att = spool.tile([128, C], DTR, tag="att")
            nc.vector.tensor_scalar_mul(att[0:64, :], e_sb[0:64, :], red[0:64, 2:3])

            # ---- final cyclic window mean + store ----
            po = papool.tile([128, C], DT, tag="pa")
            nc.tensor.matmul(
                po[0:64, :], wfin_sb[0:64, :], att[0:64, :], start=True, stop=True
            )
            ob = spool.tile([128, C], DT, tag="ob")
            nc.scalar.copy(ob[0:64, :], po[0:64, :])
            nc.sync.dma_start(out_d[:], ob[0:64, :])

    return nc


def run(inputs: dict, trace: bool = False, tmpdir: str | None = None):
    """Returns (full_output [64,256] f32, exec_time_ns or None)."""
    wn = int(np.asarray(inputs["windows"]))
    x = np.ascontiguousarray(np.asarray(inputs["x"], np.float32)).reshape(B, S, C)

    mwin = _win_matrix(wn)
    mt = np.ascontiguousarray(mwin.T)
    # partition packing p = 2*b + h: wblk[2j+h, 2i+h'] = Mt[j,i] iff h==h'
    wblk = np.zeros((128, 128), np.float32)
    wstk = np.zeros((128, 64), np.float32)
    for h in range(2):
        wblk[h::2, h::2] = mt
        wstk[h::2, :] = mt
    eye = np.eye(64, dtype=np.float32)

    common = {
        "wblk": wblk,
        "wstk": np.ascontiguousarray(wstk),
        "wfin": mt,
        "eye64": eye,
        "w1m": np.asarray(inputs["W1_max"], np.float32),
        "b1m": np.asarray(inputs["b1_max"], np.float32).reshape(CE),
        "w2m": np.asarray(inputs["W2_max"], np.float32),
        "b2m": np.asarray(inputs["b2_max"], np.float32).reshape(1, C),
        # avg branch consumes the raw spatial SUM; fold the 1/S into W1_avg
        "w1a": np.asarray(inputs["W1_avg"], np.float32) / np.float32(S),
        "b1a": np.asarray(inputs["b1_avg"], np.float32).reshape(CE),
        "w2a": np.asarray(inputs["W2_avg"], np.float32),
        "b2a": np.asarray(inputs["b2_avg"], np.float32).reshape(1, C),
    }
    in_maps = []
    for k in range(NCORES):
        m = dict(common)
        m["xs"] = np.ascontiguousarray(x[:, k * S_CORE:(k + 1) * S_CORE, :])
        in_maps.append(m)

    nc = _build_program(wn)
    nc.compile()
    res = run_bass_kernel_spmd(
        nc, in_maps, list(range(NCORES)), trace=trace, tmpdir=tmpdir,
    )
    out = np.asarray(res.results[0]["out"], np.float32)
    return out, res.exec_time_ns


def kernel(**inputs) -> np.ndarray:
    out, _ = run(inputs, trace=False)
    return out


# revision 20
# speedup vs baseline: 1.0415x; 1.0415x over previous
"""Trainium2 Bass kernel for nn_ChannelAttentionModule (cyclic window mean +
channel attention). Self-contained: accepts FULL inputs, shards spatial dim
across 8 NeuronCores, returns FULL [64, 256] output.

Math: cyclic_window_mean over the batch axis is a matmul with a fixed [64,64]
window matrix M.  Per core (spatial shard of 512 of the 4096 positions):
  - stream x as [128, 2048] tiles (partitions = 2 spatial half-groups x 64 batch)
  - PE: y = blockdiag(M^T, M^T)^T @ x_tile  -> windowed means per position
  - DVE: running elementwise max over tiles    (-> spatial max of y)
  - PE: accumulate vstack(M^T, M^T)^T @ x_tile into one PSUM bank
        (-> spatial sum of y, partition halves pre-folded)
  - AllGather the packed [64, 512] (max | sum) partials across 8 cores
  - every core folds + computes the tiny MLP / softmax / final window matmul
"""

import os
import sys

import numpy as np

for _p in ("/opt/trn_rl_repo", "/root/.axon_site/_ro/trn_rl_repo"):
    if os.path.isdir(_p) and _p not in sys.path:
        sys.path.insert(0, _p)

import concourse.bass as bass
import concourse.mybir as mybir
import concourse.tile as tile
from concourse import bacc
from concourse import bass_utils as _bu
from concourse.bass_utils import run_bass_kernel_spmd

# Redundant-LDWEIGHTS elision: every streaming matmul reuses the same
# stationary weights, so let walrus's ldw-opt pass drop the reloads.
_orig_run_command = _bu.run_command

def _run_command_ldwopt(argv, **kwargs):
    argv = [a.replace("--enable-ldw-opt=false", "--enable-ldw-opt=true")
            if isinstance(a, str) else a for a in argv]
    return _orig_run_command(argv, **kwargs)

_bu.run_command = _run_command_ldwopt

B = 64          # batch
S = 64 * 64     # flattened spatial
C = 256         # channels
CE = 768        # hidden (C * 3)
NCORES = 8
S_CORE = S // NCORES   # 512 spatial positions per core
G = 16                 # positions per partition half-group per tile
TP = 2 * G             # 16 positions per tile
NT = S_CORE // TP      # 32 tiles per core
F = G * C              # 2048 free elements per tile
FQ = 512               # matmul moving free dim (one PSUM bank, fp32)
NQ = F // FQ           # 4 matmul chunks per tile
DT = mybir.dt.float32
AF = mybir.ActivationFunctionType


def _win_matrix(w: int) -> np.ndarray:
    """M such that cyclic_window_mean(x, w) == M @ x (along axis 0)."""
    m = np.zeros((B, B), np.float64)
    for i in range(B):
        if i >= w:
            m[i, i - w:i] = 1.0 / w
        else:
            m[i, : i + 1] = 1.0 / (w + 1)
            m[i, B - (w - i):] = 1.0 / (w + 1)
    return m.astype(np.float32)


def _build_program(wn: int) -> bass.Bass:
    nc = bacc.Bacc(
        "TRN2", target_bir_lowering=False, debug=False, num_devices=NCORES
    )

    DTR = mybir.dt.float32r  # fp32 bits, single-pass PE mode (1 cyc/row)
    xs = nc.declare_dram_parameter("xs", [B, S_CORE, C], DTR, isOutput=False)
    wblk_d = nc.declare_dram_parameter("wblk", [128, 128], DTR, isOutput=False)
    wstk_d = nc.declare_dram_parameter("wstk", [128, 64], DTR, isOutput=False)
    wfin_d = nc.declare_dram_parameter("wfin", [64, 64], DTR, isOutput=False)
    eye_d = nc.declare_dram_parameter("eye64", [64, 64], DT, isOutput=False)
    w1m_d = nc.declare_dram_parameter("w1m", [C, CE], DTR, isOutput=False)
    b1m_d = nc.declare_dram_parameter("b1m", [CE], DT, isOutput=False)
    w2m_d = nc.declare_dram_parameter("w2m", [CE, C], DTR, isOutput=False)
    b2m_d = nc.declare_dram_parameter("b2m", [1, C], DT, isOutput=False)
    w1a_d = nc.declare_dram_parameter("w1a", [C, CE], DTR, isOutput=False)
    b1a_d = nc.declare_dram_parameter("b1a", [CE], DT, isOutput=False)
    w2a_d = nc.declare_dram_parameter("w2a", [CE, C], DTR, isOutput=False)
    b2a_d = nc.declare_dram_parameter("b2a", [1, C], DT, isOutput=False)
    out_d = nc.declare_dram_parameter("out", [B, C], DT, isOutput=True)

    with tile.TileContext(nc) as tc:
        with (
            tc.tile_pool(name="const", bufs=1) as cpool,
            tc.tile_pool(name="x", bufs=5) as xpool,
            tc.tile_pool(name="pmax", bufs=2, space="PSUM") as pmax,
            tc.tile_pool(name="psum_sum", bufs=1, space="PSUM") as psump,
            tc.tile_pool(name="ptail", bufs=2, space="PSUM") as ptail,
            tc.tile_pool(name="pa_pool", bufs=1, space="PSUM") as papool,
            tc.tile_pool(name="sb", bufs=1) as spool,
            tc.tile_pool(name="dram", bufs=1, space="DRAM") as dpool,
        ):
            # ---- constants into SBUF (all 128-partition tiles: matmul
            #      operands must sit at base partition 0) ----
            wblk_sb = cpool.tile([128, 128], DTR, tag="wblk")
            nc.sync.dma_start(wblk_sb[:], wblk_d[:])
            wstk_sb = cpool.tile([128, 64], DTR, tag="wstk")
            nc.sync.dma_start(wstk_sb[:], wstk_d[:])
            wfin_sb = cpool.tile([128, 64], DTR, tag="wfin")
            nc.gpsimd.dma_start(wfin_sb[0:64, :], wfin_d[:])
            eye_sb = cpool.tile([128, 64], DT, tag="eye")
            nc.gpsimd.dma_start(eye_sb[0:64, :], eye_d[:])
            # W1 as [128, 2, CE]: partition = k-within-half, free = (half, n)
            w1m_sb = cpool.tile([128, 2, CE], DTR, tag="w1m")
            nc.gpsimd.dma_start(w1m_sb[:], w1m_d[:].rearrange("(h k) n -> k h n", h=2))
            w1a_sb = cpool.tile([128, 2, CE], DTR, tag="w1a")
            nc.gpsimd.dma_start(w1a_sb[:], w1a_d[:].rearrange("(h k) n -> k h n", h=2))
            # W2 as [128, 6, C]: partition = k-within-chunk, free = (chunk, n)
            w2m_sb = cpool.tile([128, 6, C], DTR, tag="w2m")
            nc.gpsimd.dma_start(w2m_sb[:], w2m_d[:].rearrange("(m k) n -> k m n", m=6))
            w2a_sb = cpool.tile([128, 6, C], DTR, tag="w2a")
            nc.gpsimd.dma_start(w2a_sb[:], w2a_d[:].rearrange("(m k) n -> k m n", m=6))
            # b1 transposed to [128, 6] -> per-partition bias for the ACT relu
            b1m_sb = cpool.tile([128, 6], DT, tag="b1m")
            nc.gpsimd.dma_start(b1m_sb[:], b1m_d[:].rearrange("(m k) -> k m", m=6))
            b1a_sb = cpool.tile([128, 6], DT, tag="b1a")
            nc.gpsimd.dma_start(b1a_sb[:], b1a_d[:].rearrange("(m k) -> k m", m=6))
            # row-0 smalls: ones[0:64] | b2m[64:320] | b2a[320:576]
            smalls = cpool.tile([128, 576], DT, tag="smalls")
            nc.vector.memset(smalls[0:1, 0:64], 1.0)
            nc.gpsimd.dma_start(smalls[0:1, 64:320], b2m_d[:])
            nc.gpsimd.dma_start(smalls[0:1, 320:576], b2a_d[:])
            ones_sb = smalls[0:1, 0:64]
            b2m_sb = smalls[0:1, 64:320]
            b2a_sb = smalls[0:1, 320:576]

            # ---- main streaming phase ----
            # Partition packing p = 2*b + h (h = spatial half-group INNER):
            # one DMA per tile covers all 128 partitions with a flat outer
            # batch dim, so descriptors spread over all 16 SDMA engines
            # (engine = partition/8; a 64-partition half engages only 8).
            max_acc = spool.tile([128, 2 * FQ], DT, tag="max_acc")
            nc.vector.memset(max_acc[:], -1e30)
            sum_ps = psump.tile([128, FQ], DT, tag="sum_ps")
            wblk_r = wblk_sb[:]
            wstk_r = wstk_sb[:]

            # s_local = t*TP + h*G + g ; partition = (b, h) ; free = (g, c)
            xs_r = xs[:].rearrange("b (t h g) c -> t b h (g c)", h=2, g=G)
            for t in range(NT):
                xt = xpool.tile([128, F], DTR, tag="xt")
                eng = nc.sync if t % 2 == 0 else nc.scalar
                eng.dma_start(xt[:], xs_r[t])
                xt_r = xt[:]
                for half in range(F // (2 * FQ)):
                    ym = pmax.tile([128, 2 * FQ], DT, tag="ym")
                    for q in range(2):
                        col = (2 * half + q) * FQ
                        nc.tensor.matmul(
                            ym[:, q * FQ:(q + 1) * FQ], wblk_r,
                            xt_r[:, col:col + FQ], start=True, stop=True,
                        )
                    nc.vector.tensor_max(max_acc[:], max_acc[:], ym[:])
                    for q in range(2):
                        col = (2 * half + q) * FQ
                        first = (t == 0 and half == 0 and q == 0)
                        last = (t == NT - 1
                                and half == F // (2 * FQ) - 1 and q == 1)
                        nc.tensor.matmul(
                            sum_ps[0:64, :], wstk_r, xt_r[:, col:col + FQ],
                            start=first, stop=last, skip_group_check=True,
                        )

            # ---- fold local partials to [64, C] each, pack as [64, 2C] ----
            # partition fold pairs (2i, 2i+1): extract strided halves via DMA
            ev64 = spool.tile([128, 2 * FQ], DT, tag="ev64")
            od64 = spool.tile([128, 2 * FQ], DT, tag="od64")
            nc.sync.dma_start(ev64[0:64, :], max_acc[0:128:2, :])
            nc.scalar.dma_start(od64[0:64, :], max_acc[1:128:2, :])
            u = spool.tile([128, 2 * FQ], DT, tag="u")
            nc.vector.tensor_max(u[0:64, :], ev64[0:64, :], od64[0:64, :])
            nc.vector.tensor_max(u[0:64, 0:FQ], u[0:64, 0:FQ], u[0:64, FQ:2 * FQ])
            pk = spool.tile([128, 2 * C], DT, tag="pk")
            nc.vector.tensor_max(pk[0:64, 0:C], u[0:64, 0:C], u[0:64, C:2 * C])
            su = spool.tile([128, FQ], DT, tag="su")
            nc.scalar.copy(su[0:64, :], sum_ps[0:64, :])
            nc.vector.tensor_add(
                pk[0:64, C:2 * C], su[0:64, 0:C], su[0:64, C:2 * C]
            )

            # ---- cross-core combine: one AllGather + local fold ----
            rg = [list(range(NCORES))]
            gin = dpool.tile([64, 2 * C], DT, tag="gin")
            gout = dpool.tile([NCORES * 64, 2 * C], DT, tag="gout")
            nc.gpsimd.dma_start(gin[:], pk[0:64, :])
            nc.gpsimd.collective_compute(
                "AllGather", mybir.AluOpType.bypass, replica_groups=rg,
                ins=[gin.opt()], outs=[gout.opt()],
            )
            g_sb = spool.tile([128, NCORES, 2 * C], DT, tag="g_sb")
            nc.gpsimd.dma_start(
                g_sb[0:64, :, :], gout[:].rearrange("(r b) n -> b r n", r=NCORES)
            )
            mxf = spool.tile([128, C], DT, tag="mxf")
            svf = spool.tile([128, C], DT, tag="svf")
            nc.vector.tensor_copy(mxf[0:64, :], g_sb[0:64, 0, 0:C])
            nc.vector.tensor_copy(svf[0:64, :], g_sb[0:64, 0, C:2 * C])
            for r in range(1, NCORES):
                nc.vector.tensor_max(mxf[0:64, :], mxf[0:64, :], g_sb[0:64, r, 0:C])
                nc.vector.tensor_add(
                    svf[0:64, :], svf[0:64, :], g_sb[0:64, r, C:2 * C]
                )

            # ---- transpose [64, 256] -> [128, 2, 64] (chunked over C) ----
            def transpose_bc(src, tag):
                dst = spool.tile([128, 2, 64], DTR, tag=tag)
                for ch in range(2):
                    pt = ptail.tile([128, 64], DT, tag="ph")
                    nc.tensor.transpose(
                        pt[:], src[0:64, ch * 128:(ch + 1) * 128], eye_sb[0:64, :]
                    )
                    nc.scalar.copy(dst[:, ch, :], pt[:])
                return dst

            mxT = transpose_bc(mxf, "mxT")
            svT = transpose_bc(svf, "svT")

            # ---- the two tiny MLPs (relu(relu(v @ W1 + b1) @ W2 + b2)) ----
            def mlp(vT, w1_sb, b1_sb, w2_sb, b2_sb, tag):
                h1 = spool.tile([128, 6, 64], DTR, tag=f"h1_{tag}")
                for m in range(6):
                    ph = ptail.tile([128, 64], DT, tag="ph")
                    nc.tensor.matmul(
                        ph[:], w1_sb[:, 0, m * 128:(m + 1) * 128], vT[:, 0, :],
                        start=True, stop=False,
                    )
                    nc.tensor.matmul(
                        ph[:], w1_sb[:, 1, m * 128:(m + 1) * 128], vT[:, 1, :],
                        start=False, stop=True,
                    )
                    nc.scalar.activation(
                        h1[:, m, :], ph[:], AF.Relu, bias=b1_sb[:, m:m + 1]
                    )
                pa = papool.tile([128, C], DT, tag="pa")
                for m in range(6):
                    nc.tensor.matmul(
                        pa[0:64, :], h1[:, m, :], w2_sb[:, m, :],
                        start=(m == 0), stop=False,
                    )
                nc.tensor.matmul(
                    pa[0:64, :], ones_sb, b2_sb, start=False, stop=True
                )
                dst = spool.tile([128, C], DT, tag=f"mlp_{tag}")
                nc.scalar.activation(dst[0:64, :], pa[0:64, :], AF.Relu)
                return dst

            m_sb = mlp(mxT, w1m_sb, b1m_sb, w2m_sb, b2m_sb, "m")
            a_sb = mlp(svT, w1a_sb, b1a_sb, w2a_sb, b2a_sb, "a")

            # ---- sigmoid(m + a), softmax over channels ----
            t_sb = spool.tile([128, C], DT, tag="t_sb")
            nc.vector.tensor_add(t_sb[0:64, :], m_sb[0:64, :], a_sb[0:64, :])
            s_sb = spool.tile([128, C], DT, tag="s_sb")
            nc.scalar.activation(s_sb[0:64, :], t_sb[0:64, :], AF.Sigmoid)
            red = spool.tile([128, 4], DT, tag="red")  # rsum | rinv
            e_sb = spool.tile([128, C], DT, tag="e_sb")
            nc.scalar.activation(e_sb[0:64, :], s_sb[0:64, :], AF.Exp)
            nc.vector.tensor_reduce(
                red[0:64, 1:2], e_sb[0:64, :], axis=mybir.AxisListType.X,
                op=mybir.AluOpType.add,
            )
            nc.vector.reciprocal(red[0:64, 2:3], red[0:64, 1:2])
            att = spool.tile([128, C], DTR, tag="att")
            nc.vector.tensor_scalar_mul(att[0:64, :], e_sb[0:64, :], red[0:64, 2:3])

            # ---- final cyclic window mean + store ----
            po = papool.tile([128, C], DT, tag="pa")
            nc.tensor.matmul(
                po[0:64, :], wfin_sb[0:64, :], att[0:64, :], start=True, stop=True
            )
            ob = spool.tile([128, C], DT, tag="ob")
            nc.scalar.copy(ob[0:64, :], po[0:64, :])
            nc.sync.dma_start(out_d[:], ob[0:64, :])

    return nc


def run(inputs: dict, trace: bool = False, tmpdir: str | None = None):
    """Returns (full_output [64,256] f32, exec_time_ns or None)."""
    wn = int(np.asarray(inputs["windows"]))
    x = np.ascontiguousarray(np.asarray(inputs["x"], np.float32)).reshape(B, S, C)

    mwin = _win_matrix(wn)
    mt = np.ascontiguousarray(mwin.T)
    # partition packing p = 2*b + h: wblk[2j+h, 2i+h'] = Mt[j,i] iff h==h'
    wblk = np.zeros((128, 128), np.float32)
    wstk = np.zeros((128, 64), np.float32)
    for h in range(2):
        wblk[h::2, h::2] = mt
        wstk[h::2, :] = mt
    eye = np.eye(64, dtype=np.float32)

    common = {
        "wblk": wblk,
        "wstk": np.ascontiguousarray(wstk),
        "wfin": mt,
        "eye64": eye,
        "w1m": np.asarray(inputs["W1_max"], np.float32),
        "b1m": np.asarray(inputs["b1_max"], np.float32).reshape(CE),
        "w2m": np.asarray(inputs["W2_max"], np.float32),
        "b2m": np.asarray(inputs["b2_max"], np.float32).reshape(1, C),
        # avg branch consumes the raw spatial SUM; fold the 1/S into W1_avg
        "w1a": np.asarray(inputs["W1_avg"], np.float32) / np.float32(S),
        "b1a": np.asarray(inputs["b1_avg"], np.float32).reshape(CE),
        "w2a": np.asarray(inputs["W2_avg"], np.float32),
        "b2a": np.asarray(inputs["b2_avg"], np.float32).reshape(1, C),
    }
    in_maps = []
    for k in range(NCORES):
        m = dict(common)
        m["xs"] = np.ascontiguousarray(x[:, k * S_CORE:(k + 1) * S_CORE, :])
        in_maps.append(m)

    nc = _build_program(wn)
    nc.compile()
    res = run_bass_kernel_spmd(
        nc, in_maps, list(range(NCORES)), trace=trace, tmpdir=tmpdir,
    )
    out = np.asarray(res.results[0]["out"], np.float32)
    return out, res.exec_time_ns


def kernel(**inputs) -> np.ndarray:
    out, _ = run(inputs, trace=False)
    return out


# revision 23
# speedup vs baseline: 1.3117x; 1.2594x over previous
"""Trainium2 Bass kernel for nn_ChannelAttentionModule (cyclic window mean +
channel attention). Self-contained: accepts FULL inputs, shards spatial dim
across 8 NeuronCores, returns FULL [64, 256] output.

Math: cyclic_window_mean over the batch axis is a matmul with a fixed [64,64]
window matrix M.  Per core (spatial shard of 512 of the 4096 positions):
  - stream x as [128, 2048] tiles (partitions = 2 spatial half-groups x 64 batch)
  - PE: y = blockdiag(M^T, M^T)^T @ x_tile  -> windowed means per position
  - DVE: running elementwise max over tiles    (-> spatial max of y)
  - PE: accumulate vstack(M^T, M^T)^T @ x_tile into one PSUM bank
        (-> spatial sum of y, partition halves pre-folded)
  - AllGather the packed [64, 512] (max | sum) partials across 8 cores
  - every core folds + computes the tiny MLP / softmax / final window matmul
"""

import os
import sys

import numpy as np

for _p in ("/opt/trn_rl_repo", "/root/.axon_site/_ro/trn_rl_repo"):
    if os.path.isdir(_p) and _p not in sys.path:
        sys.path.insert(0, _p)

import concourse.bass as bass
import concourse.mybir as mybir
import concourse.tile as tile
from concourse import bacc
from concourse import bass_utils as _bu
from concourse.bass_utils import run_bass_kernel_spmd

# Redundant-LDWEIGHTS elision: every streaming matmul reuses the same
# stationary weights, so let walrus's ldw-opt pass drop the reloads.
_orig_run_command = _bu.run_command

def _run_command_ldwopt(argv, **kwargs):
    argv = [a.replace("--enable-ldw-opt=false", "--enable-ldw-opt=true")
            if isinstance(a, str) else a for a in argv]
    return _orig_run_command(argv, **kwargs)

# ldw-opt breaks bf16 ldweights lowering; leave disabled
# _bu.run_command = _run_command_ldwopt

B = 64          # batch
S = 64 * 64     # flattened spatial
C = 256         # channels
CE = 768        # hidden (C * 3)
NCORES = 8
S_CORE = S // NCORES   # 512 spatial positions per core
G = 32                 # positions per partition half-group per tile
TP = 2 * G             # 16 positions per tile
NT = S_CORE // TP      # 32 tiles per core
F = G * C              # 2048 free elements per tile
FQ = 512               # matmul moving free dim (one PSUM bank, fp32)
NQ = F // FQ           # 4 matmul chunks per tile
DT = mybir.dt.float32
AF = mybir.ActivationFunctionType


def _win_matrix(w: int) -> np.ndarray:
    """M such that cyclic_window_mean(x, w) == M @ x (along axis 0)."""
    m = np.zeros((B, B), np.float64)
    for i in range(B):
        if i >= w:
            m[i, i - w:i] = 1.0 / w
        else:
            m[i, : i + 1] = 1.0 / (w + 1)
            m[i, B - (w - i):] = 1.0 / (w + 1)
    return m.astype(np.float32)


def _build_program(wn: int) -> bass.Bass:
    nc = bacc.Bacc(
        "TRN2", target_bir_lowering=False, debug=False, num_devices=NCORES
    )

    DTR = mybir.dt.float32r  # fp32 bits, single-pass PE mode (1 cyc/row)
    DTB = mybir.dt.bfloat16
    xs = nc.declare_dram_parameter("xs", [B, S_CORE, C], DTB, isOutput=False)
    wblk_d = nc.declare_dram_parameter("wblk", [128, 128], DTB, isOutput=False)
    wstk_d = nc.declare_dram_parameter("wstk", [128, 64], DTB, isOutput=False)
    wsc_d = nc.declare_dram_parameter("wsc", [64, 1], DT, isOutput=False)
    wfin_d = nc.declare_dram_parameter("wfin", [64, 64], DTR, isOutput=False)
    eye_d = nc.declare_dram_parameter("eye64", [64, 64], DT, isOutput=False)
    w1m_d = nc.declare_dram_parameter("w1m", [C, CE], DTR, isOutput=False)
    b1m_d = nc.declare_dram_parameter("b1m", [CE], DT, isOutput=False)
    w2m_d = nc.declare_dram_parameter("w2m", [CE, C], DTR, isOutput=False)
    b2m_d = nc.declare_dram_parameter("b2m", [1, C], DT, isOutput=False)
    w1a_d = nc.declare_dram_parameter("w1a", [C, CE], DTR, isOutput=False)
    b1a_d = nc.declare_dram_parameter("b1a", [CE], DT, isOutput=False)
    w2a_d = nc.declare_dram_parameter("w2a", [CE, C], DTR, isOutput=False)
    b2a_d = nc.declare_dram_parameter("b2a", [1, C], DT, isOutput=False)
    out_d = nc.declare_dram_parameter("out", [B, C], DT, isOutput=True)

    with tile.TileContext(nc) as tc:
        with (
            tc.tile_pool(name="const", bufs=1) as cpool,
            tc.tile_pool(name="x", bufs=3) as xpool,
            tc.tile_pool(name="pmax", bufs=2, space="PSUM") as pmax,
            tc.tile_pool(name="psum_sum", bufs=1, space="PSUM") as psump,
            tc.tile_pool(name="ptail", bufs=2, space="PSUM") as ptail,
            tc.tile_pool(name="pa_pool", bufs=1, space="PSUM") as papool,
            tc.tile_pool(name="sb", bufs=1) as spool,
            tc.tile_pool(name="dram", bufs=1, space="DRAM") as dpool,
        ):
            # ---- constants into SBUF (all 128-partition tiles: matmul
            #      operands must sit at base partition 0) ----
            wblk_sb = cpool.tile([128, 128], DTB, tag="wblk")
            nc.sync.dma_start(wblk_sb[:], wblk_d[:])
            wstk_sb = cpool.tile([128, 64], DTB, tag="wstk")
            nc.sync.dma_start(wstk_sb[:], wstk_d[:])
            wfin_sb = cpool.tile([128, 64], DTR, tag="wfin")
            nc.gpsimd.dma_start(wfin_sb[0:64, :], wfin_d[:])
            wsc_sb = cpool.tile([128, 1], DT, tag="wsc")
            nc.gpsimd.dma_start(wsc_sb[0:64, :], wsc_d[:])
            eye_sb = cpool.tile([128, 64], DT, tag="eye")
            nc.gpsimd.dma_start(eye_sb[0:64, :], eye_d[:])
            # W1 as [128, 2, CE]: partition = k-within-half, free = (half, n)
            w1m_sb = cpool.tile([128, 2, CE], DTR, tag="w1m")
            nc.gpsimd.dma_start(w1m_sb[:], w1m_d[:].rearrange("(h k) n -> k h n", h=2))
            w1a_sb = cpool.tile([128, 2, CE], DTR, tag="w1a")
            nc.gpsimd.dma_start(w1a_sb[:], w1a_d[:].rearrange("(h k) n -> k h n", h=2))
            # W2 as [128, 6, C]: partition = k-within-chunk, free = (chunk, n)
            w2m_sb = cpool.tile([128, 6, C], DTR, tag="w2m")
            nc.gpsimd.dma_start(w2m_sb[:], w2m_d[:].rearrange("(m k) n -> k m n", m=6))
            w2a_sb = cpool.tile([128, 6, C], DTR, tag="w2a")
            nc.gpsimd.dma_start(w2a_sb[:], w2a_d[:].rearrange("(m k) n -> k m n", m=6))
            # b1 transposed to [128, 6] -> per-partition bias for the ACT relu
            b1m_sb = cpool.tile([128, 6], DT, tag="b1m")
            nc.gpsimd.dma_start(b1m_sb[:], b1m_d[:].rearrange("(m k) -> k m", m=6))
            b1a_sb = cpool.tile([128, 6], DT, tag="b1a")
            nc.gpsimd.dma_start(b1a_sb[:], b1a_d[:].rearrange("(m k) -> k m", m=6))
            # row-0 smalls: ones[0:64] | b2m[64:320] | b2a[320:576]
            smalls = cpool.tile([128, 576], DT, tag="smalls")
            nc.vector.memset(smalls[0:1, 0:64], 1.0)
            nc.gpsimd.dma_start(smalls[0:1, 64:320], b2m_d[:])
            nc.gpsimd.dma_start(smalls[0:1, 320:576], b2a_d[:])
            ones_sb = smalls[0:1, 0:64]
            b2m_sb = smalls[0:1, 64:320]
            b2a_sb = smalls[0:1, 320:576]

            # ---- main streaming phase ----
            # Partition packing p = 2*b + h (h = spatial half-group INNER):
            # one DMA per tile covers all 128 partitions with a flat outer
            # batch dim, so descriptors spread over all 16 SDMA engines
            # (engine = partition/8; a 64-partition half engages only 8).
            max_acc = spool.tile([128, 2 * FQ], DT, tag="max_acc")
            nc.vector.memset(max_acc[:], -1e30)
            sum_ps = psump.tile([128, FQ], DT, tag="sum_ps")
            wblk_r = wblk_sb[:]
            wstk_r = wstk_sb[:]

            # s_local = t*TP + h*G + g ; partition = (b, h) ; free = (g, c)
            xs_r = xs[:].rearrange("b (t h g) c -> t b h (g c)", h=2, g=G)
            for t in range(NT):
                xt = xpool.tile([128, F], DTB, tag="xt")
                eng = nc.sync if t % 2 == 0 else nc.scalar
                eng.dma_start(xt[:], xs_r[t])
                xt_r = xt[:]
                for half in range(F // (2 * FQ)):
                    ym = pmax.tile([128, 2 * FQ], DT, tag="ym")
                    for q in range(2):
                        col = (2 * half + q) * FQ
                        nc.tensor.matmul(
                            ym[:, q * FQ:(q + 1) * FQ], wblk_r,
                            xt_r[:, col:col + FQ], start=True, stop=True,
                        )
                    nc.vector.tensor_max(max_acc[:], max_acc[:], ym[:])
                    for q in range(2):
                        col = (2 * half + q) * FQ
                        first = (t == 0 and half == 0 and q == 0)
                        last = (t == NT - 1
                                and half == F // (2 * FQ) - 1 and q == 1)
                        nc.tensor.matmul(
                            sum_ps[0:64, :], wstk_r, xt_r[:, col:col + FQ],
                            start=first, stop=last, skip_group_check=True,
                        )

            # ---- fold local partials to [64, C] each, pack as [64, 2C] ----
            # partition fold pairs (2i, 2i+1): extract strided halves via DMA
            ev64 = spool.tile([128, 2 * FQ], DT, tag="ev64")
            od64 = spool.tile([128, 2 * FQ], DT, tag="od64")
            nc.sync.dma_start(ev64[0:64, :], max_acc[0:128:2, :])
            nc.scalar.dma_start(od64[0:64, :], max_acc[1:128:2, :])
            u = spool.tile([128, 2 * FQ], DT, tag="u")
            nc.vector.tensor_max(u[0:64, :], ev64[0:64, :], od64[0:64, :])
            nc.vector.tensor_max(u[0:64, 0:FQ], u[0:64, 0:FQ], u[0:64, FQ:2 * FQ])
            pk = spool.tile([128, 2 * C], DT, tag="pk")
            nc.vector.tensor_max(pk[0:64, 0:C], u[0:64, 0:C], u[0:64, C:2 * C])
            su = spool.tile([128, FQ], DT, tag="su")
            nc.scalar.copy(su[0:64, :], sum_ps[0:64, :])
            nc.vector.tensor_add(
                pk[0:64, C:2 * C], su[0:64, 0:C], su[0:64, C:2 * C]
            )

            # ---- cross-core combine: one AllGather + local fold ----
            rg = [list(range(NCORES))]
            gin = dpool.tile([64, 2 * C], DT, tag="gin")
            gout = dpool.tile([NCORES * 64, 2 * C], DT, tag="gout")
            nc.gpsimd.dma_start(gin[:], pk[0:64, :])
            nc.gpsimd.collective_compute(
                "AllGather", mybir.AluOpType.bypass, replica_groups=rg,
                ins=[gin.opt()], outs=[gout.opt()],
            )
            g_sb = spool.tile([128, NCORES, 2 * C], DT, tag="g_sb")
            nc.gpsimd.dma_start(
                g_sb[0:64, :, :], gout[:].rearrange("(r b) n -> b r n", r=NCORES)
            )
            mxf = spool.tile([128, C], DT, tag="mxf")
            svf = spool.tile([128, C], DT, tag="svf")
            nc.vector.tensor_copy(mxf[0:64, :], g_sb[0:64, 0, 0:C])
            nc.vector.tensor_copy(svf[0:64, :], g_sb[0:64, 0, C:2 * C])
            for r in range(1, NCORES):
                nc.vector.tensor_max(mxf[0:64, :], mxf[0:64, :], g_sb[0:64, r, 0:C])
                nc.vector.tensor_add(
                    svf[0:64, :], svf[0:64, :], g_sb[0:64, r, C:2 * C]
                )

            # window sums -> window means: scale rows by 1/w_i
            nc.vector.tensor_scalar_mul(mxf[0:64, :], mxf[0:64, :], wsc_sb[0:64, :])
            nc.vector.tensor_scalar_mul(svf[0:64, :], svf[0:64, :], wsc_sb[0:64, :])

            # ---- transpose [64, 256] -> [128, 2, 64] (chunked over C) ----
            def transpose_bc(src, tag):
                dst = spool.tile([128, 2, 64], DTR, tag=tag)
                for ch in range(2):
                    pt = ptail.tile([128, 64], DT, tag="ph")
                    nc.tensor.transpose(
                        pt[:], src[0:64, ch * 128:(ch + 1) * 128], eye_sb[0:64, :]
                    )
                    nc.scalar.copy(dst[:, ch, :], pt[:])
                return dst

            mxT = transpose_bc(mxf, "mxT")
            svT = transpose_bc(svf, "svT")

            # ---- the two tiny MLPs (relu(relu(v @ W1 + b1) @ W2 + b2)) ----
            def mlp(vT, w1_sb, b1_sb, w2_sb, b2_sb, tag):
                h1 = spool.tile([128, 6, 64], DTR, tag=f"h1_{tag}")
                for m in range(6):
                    ph = ptail.tile([128, 64], DT, tag="ph")
                    nc.tensor.matmul(
                        ph[:], w1_sb[:, 0, m * 128:(m + 1) * 128], vT[:, 0, :],
                        start=True, stop=False,
                    )
                    nc.tensor.matmul(
                        ph[:], w1_sb[:, 1, m * 128:(m + 1) * 128], vT[:, 1, :],
                        start=False, stop=True,
                    )
                    nc.scalar.activation(
                        h1[:, m, :], ph[:], AF.Relu, bias=b1_sb[:, m:m + 1]
                    )
                pa = papool.tile([128, C], DT, tag="pa")
                for m in range(6):
                    nc.tensor.matmul(
                        pa[0:64, :], h1[:, m, :], w2_sb[:, m, :],
                        start=(m == 0), stop=False,
                    )
                nc.tensor.matmul(
                    pa[0:64, :], ones_sb, b2_sb, start=False, stop=True
                )
                dst = spool.tile([128, C], DT, tag=f"mlp_{tag}")
                nc.scalar.activation(dst[0:64, :], pa[0:64, :], AF.Relu)
                return dst

            m_sb = mlp(mxT, w1m_sb, b1m_sb, w2m_sb, b2m_sb, "m")
            a_sb = mlp(svT, w1a_sb, b1a_sb, w2a_sb, b2a_sb, "a")

            # ---- sigmoid(m + a), softmax over channels ----
            t_sb = spool.tile([128, C], DT, tag="t_sb")
            nc.vector.tensor_add(t_sb[0:64, :], m_sb[0:64, :], a_sb[0:64, :])
            s_sb = spool.tile([128, C], DT, tag="s_sb")
            nc.scalar.activation(s_sb[0:64, :], t_sb[0:64, :], AF.Sigmoid)
            red = spool.tile([128, 4], DT, tag="red")  # rsum | rinv
            e_sb = spool.tile([128, C], DT, tag="e_sb")
            nc.scalar.activation(e_sb[0:64, :], s_sb[0:64, :], AF.Exp)
            nc.vector.tensor_reduce(
                red[0:64, 1:2], e_sb[0:64, :], axis=mybir.AxisListType.X,
                op=mybir.AluOpType.add,
            )
            nc.vector.reciprocal(red[0:64, 2:3], red[0:64, 1:2])
            att = spool.tile([128, C], DTR, tag="att")
            nc.vector.tensor_scalar_mul(att[0:64, :], e_sb[0:64, :], red[0:64, 2:3])

            # ---- final cyclic window mean + store ----
            po = papool.tile([128, C], DT, tag="pa")
            nc.tensor.matmul(
                po[0:64, :], wfin_sb[0:64, :], att[0:64, :], start=True, stop=True
            )
            ob = spool.tile([128, C], DT, tag="ob")
            nc.scalar.copy(ob[0:64, :], po[0:64, :])
            nc.sync.dma_start(out_d[:], ob[0:64, :])

    return nc


def run(inputs: dict, trace: bool = False, tmpdir: str | None = None):
    """Returns (full_output [64,256] f32, exec_time_ns or None)."""
    wn = int(np.asarray(inputs["windows"]))
    x = np.ascontiguousarray(np.asarray(inputs["x"], np.float32)).reshape(B, S, C)

    import ml_dtypes
    mwin = _win_matrix(wn)
    mt = np.ascontiguousarray(mwin.T)
    # 0/1 window-membership matrix (exact in bf16); per-row counts -> wsc
    m01 = (mwin > 0).astype(np.float32).T           # [j, i]
    cnt = (mwin > 0).sum(axis=1).astype(np.float32)  # rows of M
    wsc = (1.0 / cnt).reshape(64, 1)
    # partition packing p = 2*b + h: wblk[2j+h, 2i+h'] = m01[j,i] iff h==h'
    wblk = np.zeros((128, 128), np.float32)
    wstk = np.zeros((128, 64), np.float32)
    for h in range(2):
        wblk[h::2, h::2] = m01
        wstk[h::2, :] = m01
    wblk = wblk.astype(ml_dtypes.bfloat16)
    wstk = np.ascontiguousarray(wstk).astype(ml_dtypes.bfloat16)
    eye = np.eye(64, dtype=np.float32)

    common = {
        "wblk": wblk,
        "wstk": wstk,
        "wsc": wsc.astype(np.float32),
        "wfin": mt,
        "eye64": eye,
        "w1m": np.asarray(inputs["W1_max"], np.float32),
        "b1m": np.asarray(inputs["b1_max"], np.float32).reshape(CE),
        "w2m": np.asarray(inputs["W2_max"], np.float32),
        "b2m": np.asarray(inputs["b2_max"], np.float32).reshape(1, C),
        # avg branch consumes the raw spatial SUM; fold the 1/S into W1_avg
        "w1a": np.asarray(inputs["W1_avg"], np.float32) / np.float32(S),
        "b1a": np.asarray(inputs["b1_avg"], np.float32).reshape(CE),
        "w2a": np.asarray(inputs["W2_avg"], np.float32),
        "b2a": np.asarray(inputs["b2_avg"], np.float32).reshape(1, C),
    }
    in_maps = []
    for k in range(NCORES):
        m = dict(common)
        m["xs"] = np.ascontiguousarray(x[:, k * S_CORE:(k + 1) * S_CORE, :]).astype(ml_dtypes.bfloat16)
        in_maps.append(m)

    nc = _build_program(wn)
    nc.compile()
    res = run_bass_kernel_spmd(
        nc, in_maps, list(range(NCORES)), trace=trace, tmpdir=tmpdir,
    )
    out = np.asarray(res.results[0]["out"], np.float32)
    return out, res.exec_time_ns


def kernel(**inputs) -> np.ndarray:
    out, _ = run(inputs, trace=False)
    return out
